# revision 27
# baseline (speedup 1.0000x reference)
"""Trainium2 Bass kernel for nn_BitwiseHashing.

Computes out = tanh(mean_l(x) @ W.T + b) for x:[12,8192,1024] f32,
W:[64,1024], b:[64] -> out:[8192,64].

Strategy (data-parallel over 8 NeuronCores):
  - shard x along batch dim: 1024 rows per core (48 MiB each, streamed).
  - host pre-transposes W to wt = (W.T / L) [1024,64]; bias shipped as [1,64].
  - per 128-row block: stream 12 L-slices (contiguous 512 KiB DMAs),
    accumulate with DVE adds, PE-transpose the 8 [128,128] d-chunks of the
    sum, then project against wt.  The PSUM->SBUF copy on ACT casts the
    transposed sum to bf16 for free, so the projection matmuls run
    single-pass bf16 instead of double-pass fp32 -- that shortens the
    per-block post-arrival latency, which sets the drain tail after the
    last HBM byte lands.

  Scheduling note (measured, do not "fix"): the transposes reading the
  in-place accumulator tile deliberately tie two x-load triggers per
  block to the PE pipeline.  That acts as a governor holding the stream
  at a stable ~396 GB/s.  Decoupling the accumulator into its own pool
  lets the stream sprint at 419 GB/s (the SDMA fabric cap) but the
  free-running regime is only marginally stable and collapses to
  ~338 GB/s mid-stream, which is a net loss (measured repeatedly).
"""

import numpy as np

import concourse.bacc as bacc
import concourse.mybir as mybir
from concourse import tile
from concourse.masks import make_identity
from concourse.bass_utils import run_bass_kernel_spmd

L, B, D, K = 12, 8192, 1024, 64
NCORES = 8
BS = B // NCORES      # 1024 batch rows per core
P = 128               # partitions
NBLK = BS // P        # 8 row blocks per core
NDC = D // P          # 8 contraction chunks
F32 = mybir.dt.float32
BF16 = mybir.dt.bfloat16

_nc_cache = None


def _build():
    global _nc_cache
    if _nc_cache is not None:
        return _nc_cache

    nc = bacc.Bacc("TRN2", target_bir_lowering=False, debug=False)
    x = nc.dram_tensor("x", [L, BS, D], F32, kind="ExternalInput")
    wt = nc.dram_tensor("wt", [D, K], F32, kind="ExternalInput")
    bias = nc.dram_tensor("bias", [1, K], F32, kind="ExternalInput")
    y = nc.dram_tensor("y", [BS, K], F32, kind="ExternalOutput")

    with tile.TileContext(nc) as tc:
        with (
            tc.tile_pool(name="const", bufs=1) as cpool,
            tc.tile_pool(name="xin", bufs=26) as xpool,
            tc.tile_pool(name="xt", bufs=2) as tpool,
            tc.tile_pool(name="out", bufs=3) as opool,
            tc.tile_pool(name="pt", bufs=2, space="PSUM") as pt_pool,
            tc.tile_pool(name="po", bufs=2, space="PSUM") as po_pool,
        ):
            # constants go over the SWDGE queue to keep both HWDGE rings
            # free for the x stream from t=0; the bf16 casts run on
            # gpsimd during the startup window
            wt_f32 = cpool.tile([P, NDC * K], F32)
            for dc in range(NDC):
                nc.gpsimd.dma_start(
                    out=wt_f32[:, dc * K:(dc + 1) * K],
                    in_=wt.ap()[dc * P:(dc + 1) * P, :],
                )
            bias_f32 = cpool.tile([1, K], F32)
            nc.gpsimd.dma_start(out=bias_f32[:], in_=bias.ap())
            wt_sb = cpool.tile([P, NDC * K], BF16)
            nc.gpsimd.tensor_copy(out=wt_sb[:], in_=wt_f32[:])
            bias_sb = cpool.tile([1, K], BF16)
            nc.gpsimd.tensor_copy(out=bias_sb[:], in_=bias_f32[:])
            ones_sb = cpool.tile([1, P], BF16)
            nc.gpsimd.memset(ones_sb[:], 1.0)
            ident = cpool.tile([P, P], F32)
            make_identity(nc, ident[:])

            xap = x.ap()
            yap = y.ap()

            def issue_loads(blk):
                b0 = blk * P
                xt = []
                for l in range(L):
                    xl = xpool.tile([P, D], F32)
                    eng = nc.sync if l % 2 == 0 else nc.scalar
                    eng.dma_start(out=xl[:], in_=xap[l, b0:b0 + P, :])
                    xt.append(xl)
                return xt

            def reduce(xt):
                # two independent running chains, one per DMA ring: the
                # even tiles (sync ring) and odd tiles (scalar ring) each
                # complete in FIFO order within their ring, so each chain
                # only ever waits on its own ring and inter-ring skew
                # cannot stall the reduction
                accE, accO = xt[0], xt[1]
                for l in range(2, L, 2):
                    nc.vector.tensor_add(
                        out=accE[:], in0=accE[:], in1=xt[l][:]
                    )
                    nc.vector.tensor_add(
                        out=accO[:], in0=accO[:], in1=xt[l + 1][:]
                    )
                nc.vector.tensor_add(out=accE[:], in0=accE[:], in1=accO[:])
                return accE

            def project(acc):
                # transpose the block sum into PSUM (single-op groups),
                # then one wide PSUM->SBUF copy on ACT that also casts to
                # bf16, and the K-projection in single-pass bf16 matmuls
                pt_all = pt_pool.tile([P, D], F32)
                for dc in range(NDC):
                    nc.tensor.transpose(
                        pt_all[:, dc * P:(dc + 1) * P],
                        acc[:, dc * P:(dc + 1) * P],
                        ident[:],
                    )
                xt_all = tpool.tile([P, D], BF16)
                nc.scalar.copy(out=xt_all[:], in_=pt_all[:])

                po = po_pool.tile([P, K], F32)
                # bias broadcast across partitions: ones[1,128].T @ bias[1,64]
                nc.tensor.matmul(
                    po[:], lhsT=ones_sb[:], rhs=bias_sb[:], start=True, stop=False
                )
                for dc in range(NDC):
                    nc.tensor.matmul(
                        po[:],
                        lhsT=xt_all[:, dc * P:(dc + 1) * P],
                        rhs=wt_sb[:, dc * K:(dc + 1) * K],
                        start=False,
                        stop=(dc == NDC - 1),
                    )
                return po

            def finish(blk, po):
                b0 = blk * P
                ot = opool.tile([P, K], F32)
                nc.scalar.activation(
                    ot[:], po[:], mybir.ActivationFunctionType.Tanh
                )
                nc.sync.dma_start(out=yap[b0:b0 + P, :], in_=ot[:])

            H = D // 2

            def reduce_last(xt):
                # last block: the tail after the final HBM byte is pure
                # drain, so pipeline it in D-halves -- only ~1.2us of DVE
                # work (two half-width adds) gates the first transposes
                # instead of ~2.4us of full-width adds
                accE, accO = xt[0], xt[1]
                for l in range(2, L - 2, 2):
                    nc.vector.tensor_add(
                        out=accE[:], in0=accE[:], in1=xt[l][:]
                    )
                    nc.vector.tensor_add(
                        out=accO[:], in0=accO[:], in1=xt[l + 1][:]
                    )
                nc.vector.tensor_add(out=accE[:], in0=accE[:], in1=xt[10][:])
                nc.vector.tensor_add(
                    out=accO[:, :H], in0=accO[:, :H], in1=xt[11][:, :H]
                )
                nc.vector.tensor_add(
                    out=accE[:, :H], in0=accE[:, :H], in1=accO[:, :H]
                )
                nc.vector.tensor_add(
                    out=accO[:, H:], in0=accO[:, H:], in1=xt[11][:, H:]
                )
                nc.vector.tensor_add(
                    out=accE[:, H:], in0=accE[:, H:], in1=accO[:, H:]
                )
                return accE

            def project_last(acc):
                # half-pipelined: transposes+copy of the low 512 columns
                # start while the high half is still merging; projection
                # matmuls for the low half overlap the high half's copy
                pt_all = pt_pool.tile([P, D], F32)
                xt_all = tpool.tile([P, D], BF16)
                for dc in range(NDC // 2):
                    nc.tensor.transpose(
                        pt_all[:, dc * P:(dc + 1) * P],
                        acc[:, dc * P:(dc + 1) * P],
                        ident[:],
                    )
                nc.scalar.copy(out=xt_all[:, :H], in_=pt_all[:, :H])
                for dc in range(NDC // 2, NDC):
                    nc.tensor.transpose(
                        pt_all[:, dc * P:(dc + 1) * P],
                        acc[:, dc * P:(dc + 1) * P],
                        ident[:],
                    )
                nc.scalar.copy(out=xt_all[:, H:], in_=pt_all[:, H:])

                po = po_pool.tile([P, K], F32)
                nc.tensor.matmul(
                    po[:], lhsT=ones_sb[:], rhs=bias_sb[:], start=True, stop=False
                )
                for dc in range(NDC):
                    nc.tensor.matmul(
                        po[:],
                        lhsT=xt_all[:, dc * P:(dc + 1) * P],
                        rhs=wt_sb[:, dc * K:(dc + 1) * K],
                        start=False,
                        stop=(dc == NDC - 1),
                    )
                return po

            # Emission order per block: adds(n) -> loads(n+1) -> psum/matmul
            # stage(n) -> tanh+y(n-1). This keeps every ACT/sync DMA trigger
            # for block n+1 AHEAD of block n's copy/tanh/y in the engine
            # FIFOs, so the two x-stream rings never stall behind compute.
            # (Putting project(n) before loads(n+1) was measured to REMOVE
            # the stabilizing transpose-governor: the stream then sprints
            # and collapses to ~337 GB/s -- do not reorder.)
            xt = issue_loads(0)
            prev_po = None
            for blk in range(NBLK):
                last = blk == NBLK - 1
                acc = reduce_last(xt) if last else reduce(xt)
                if not last:
                    xt = issue_loads(blk + 1)
                po = project_last(acc) if last else project(acc)
                if prev_po is not None:
                    finish(blk - 1, prev_po)
                prev_po = po
            finish(NBLK - 1, prev_po)

    nc.compile()
    _nc_cache = nc
    return nc


def _ensure_ntff_hook():
    """Register the axon NTFF profile hook if the image's antenv lacks it."""
    import sys
    import types

    try:
        from antenv.axon_hooks import get_axon_ntff_profile_hook  # noqa: F401
        return
    except ImportError:
        pass
    import antenv

    mod = types.ModuleType("antenv.axon_hooks")
    mod._hook = None

    def set_axon_ntff_profile_hook(h):
        mod._hook = h

    def get_axon_ntff_profile_hook():
        return mod._hook

    mod.set_axon_ntff_profile_hook = set_axon_ntff_profile_hook
    mod.get_axon_ntff_profile_hook = get_axon_ntff_profile_hook
    sys.modules["antenv.axon_hooks"] = mod
    antenv.axon_hooks = mod
    try:
        from trn_agent_boot.trn_boot import _ntff_profile_via_ctypes

        mod._hook = _ntff_profile_via_ctypes("/opt/axon/libaxon_pjrt.so")
    except Exception:
        mod._hook = None


def _run(inputs, trace=False, **kwargs):
    x = np.asarray(inputs["x"], dtype=np.float32)
    W = np.asarray(inputs["W"], dtype=np.float32)
    b = np.asarray(inputs["b"], dtype=np.float32)
    wt = np.ascontiguousarray(W.T).astype(np.float32) * np.float32(1.0 / L)
    bias = np.ascontiguousarray(b.reshape(1, K)).astype(np.float32)
    in_maps = [
        {
            "x": np.ascontiguousarray(x[:, c * BS:(c + 1) * BS, :]),
            "wt": wt,
            "bias": bias,
        }
        for c in range(NCORES)
    ]
    if trace:
        _ensure_ntff_hook()
        import concourse.bass_utils as bu

        bu.upload_artifacts = lambda tmpdir: "local://skipped"
    nc = _build()
    res = run_bass_kernel_spmd(
        nc, in_maps, core_ids=list(range(NCORES)), trace=trace, **kwargs
    )
    y = np.concatenate([r["y"] for r in res.results], axis=0)
    return y, res


def kernel(**inputs):
    y, _ = _run(inputs)
    return y


# revision 29
# speedup vs baseline: 1.0478x; 1.0478x over previous
"""Trainium2 Bass kernel for nn_BitwiseHashing.

Computes out = tanh(mean_l(x) @ W.T + b) for x:[12,8192,1024] f32,
W:[64,1024], b:[64] -> out:[8192,64].

Strategy (data-parallel over 8 NeuronCores):
  - shard x along batch dim: 1024 rows per core (48 MiB each, streamed).
  - host pre-transposes W to wt = (W.T / L) [1024,64]; bias shipped as [1,64].
  - per 128-row block: stream 12 L-slices (contiguous 512 KiB DMAs),
    accumulate with DVE adds, PE-transpose the 8 [128,128] d-chunks of the
    sum, then project against wt.  The PSUM->SBUF copy on ACT casts the
    transposed sum to bf16 for free, so the projection matmuls run
    single-pass bf16 instead of double-pass fp32 -- that shortens the
    per-block post-arrival latency, which sets the drain tail after the
    last HBM byte lands.

  Scheduling note (measured, do not "fix"): the transposes reading the
  in-place accumulator tile deliberately tie two x-load triggers per
  block to the PE pipeline.  That acts as a governor holding the stream
  at a stable ~396 GB/s.  Decoupling the accumulator into its own pool
  lets the stream sprint at 419 GB/s (the SDMA fabric cap) but the
  free-running regime is only marginally stable and collapses to
  ~338 GB/s mid-stream, which is a net loss (measured repeatedly).
"""

import numpy as np

import concourse.bacc as bacc
import concourse.mybir as mybir
from concourse import tile
from concourse.masks import make_identity
from concourse.bass_utils import run_bass_kernel_spmd

L, B, D, K = 12, 8192, 1024, 64
NCORES = 8
BS = B // NCORES      # 1024 batch rows per core
P = 128               # partitions
NBLK = BS // P        # 8 row blocks per core
NDC = D // P          # 8 contraction chunks
F32 = mybir.dt.float32
BF16 = mybir.dt.bfloat16

_nc_cache = None


def _build():
    global _nc_cache
    if _nc_cache is not None:
        return _nc_cache

    nc = bacc.Bacc("TRN2", target_bir_lowering=False, debug=False)
    x = nc.dram_tensor("x", [L, BS, D], F32, kind="ExternalInput")
    wt = nc.dram_tensor("wt", [D, K], F32, kind="ExternalInput")
    bias = nc.dram_tensor("bias", [1, K], F32, kind="ExternalInput")
    y = nc.dram_tensor("y", [BS, K], F32, kind="ExternalOutput")

    with tile.TileContext(nc) as tc:
        with (
            tc.tile_pool(name="const", bufs=1) as cpool,
            tc.tile_pool(name="xin", bufs=26) as xpool,
            tc.tile_pool(name="xt", bufs=2) as tpool,
            tc.tile_pool(name="out", bufs=3) as opool,
            tc.tile_pool(name="pt", bufs=2, space="PSUM") as pt_pool,
            tc.tile_pool(name="po", bufs=2, space="PSUM") as po_pool,
        ):
            # constants go over the SWDGE queue to keep both HWDGE rings
            # free for the x stream from t=0; the bf16 casts run on
            # gpsimd during the startup window
            wt_f32 = cpool.tile([P, NDC * K], F32)
            for dc in range(NDC):
                nc.gpsimd.dma_start(
                    out=wt_f32[:, dc * K:(dc + 1) * K],
                    in_=wt.ap()[dc * P:(dc + 1) * P, :],
                )
            bias_f32 = cpool.tile([1, K], F32)
            nc.gpsimd.dma_start(out=bias_f32[:], in_=bias.ap())
            wt_sb = cpool.tile([P, NDC * K], BF16)
            nc.gpsimd.tensor_copy(out=wt_sb[:], in_=wt_f32[:])
            bias_sb = cpool.tile([1, K], BF16)
            nc.gpsimd.tensor_copy(out=bias_sb[:], in_=bias_f32[:])
            ones_sb = cpool.tile([1, P], BF16)
            nc.gpsimd.memset(ones_sb[:], 1.0)
            ident = cpool.tile([P, P], F32)
            make_identity(nc, ident[:])
            # last-block scratch: fresh single-write destinations for the
            # tail adds -- in-place sub-range accumulation there made the
            # Tile scheduler insert an ~0.9us DVE DRAIN mid-drain
            accL = cpool.tile([P, D], F32)
            accN = cpool.tile([P, D], F32)
            acch2 = cpool.tile([P, D], F32)

            xap = x.ap()
            yap = y.ap()

            def issue_loads(blk):
                b0 = blk * P
                xt = []
                for l in range(L):
                    xl = xpool.tile([P, D], F32)
                    eng = nc.sync if l % 2 == 0 else nc.scalar
                    eng.dma_start(out=xl[:], in_=xap[l, b0:b0 + P, :])
                    xt.append(xl)
                return xt

            def reduce(xt):
                # two independent running chains, one per DMA ring: the
                # even tiles (sync ring) and odd tiles (scalar ring) each
                # complete in FIFO order within their ring, so each chain
                # only ever waits on its own ring and inter-ring skew
                # cannot stall the reduction
                accE, accO = xt[0], xt[1]
                for l in range(2, L, 2):
                    nc.vector.tensor_add(
                        out=accE[:], in0=accE[:], in1=xt[l][:]
                    )
                    nc.vector.tensor_add(
                        out=accO[:], in0=accO[:], in1=xt[l + 1][:]
                    )
                nc.vector.tensor_add(out=accE[:], in0=accE[:], in1=accO[:])
                return accE

            def project(acc):
                # transpose the block sum into PSUM (single-op groups),
                # then one wide PSUM->SBUF copy on ACT that also casts to
                # bf16, and the K-projection in single-pass bf16 matmuls
                pt_all = pt_pool.tile([P, D], F32)
                for dc in range(NDC):
                    nc.tensor.transpose(
                        pt_all[:, dc * P:(dc + 1) * P],
                        acc[:, dc * P:(dc + 1) * P],
                        ident[:],
                    )
                xt_all = tpool.tile([P, D], BF16)
                nc.scalar.copy(out=xt_all[:], in_=pt_all[:])

                po = po_pool.tile([P, K], F32)
                # bias broadcast across partitions: ones[1,128].T @ bias[1,64]
                nc.tensor.matmul(
                    po[:], lhsT=ones_sb[:], rhs=bias_sb[:], start=True, stop=False
                )
                for dc in range(NDC):
                    nc.tensor.matmul(
                        po[:],
                        lhsT=xt_all[:, dc * P:(dc + 1) * P],
                        rhs=wt_sb[:, dc * K:(dc + 1) * K],
                        start=False,
                        stop=(dc == NDC - 1),
                    )
                return po

            def finish(blk, po):
                b0 = blk * P
                ot = opool.tile([P, K], F32)
                nc.scalar.activation(
                    ot[:], po[:], mybir.ActivationFunctionType.Tanh
                )
                nc.sync.dma_start(out=yap[b0:b0 + P, :], in_=ot[:])

            H = D // 2

            def reduce_last(xt):
                # last block: the tail after the final HBM byte is pure
                # drain, so pipeline it in D-halves -- only ~1.2us of DVE
                # work (two half-width adds) gates the first transposes
                # instead of ~2.4us of full-width adds
                accE, accO = xt[0], xt[1]
                for l in range(2, L - 2, 2):
                    nc.vector.tensor_add(
                        out=accE[:], in0=accE[:], in1=xt[l][:]
                    )
                    nc.vector.tensor_add(
                        out=accO[:], in0=accO[:], in1=xt[l + 1][:]
                    )
                # t10/t11 + merge at half granularity, every write to a
                # fresh tile range (single-write: no WAW/DRAIN hazards);
                # the h0 merge completes ~1.7us after the last byte and
                # unblocks the first transposes
                for h in (slice(0, H), slice(H, D)):
                    nc.vector.tensor_add(
                        out=accL[:, h], in0=accE[:, h], in1=xt[10][:, h]
                    )
                    nc.vector.tensor_add(
                        out=accN[:, h], in0=accO[:, h], in1=xt[11][:, h]
                    )
                    nc.vector.tensor_add(
                        out=acch2[:, h], in0=accL[:, h], in1=accN[:, h]
                    )
                return acch2

            def project_last(acc):
                # half-pipelined: transposes+copy of the low 512 columns
                # start while the high half is still merging; projection
                # matmuls for the low half overlap the high half's copy
                pt_all = pt_pool.tile([P, D], F32)
                xt_all = tpool.tile([P, D], BF16)
                for dc in range(NDC // 2):
                    nc.tensor.transpose(
                        pt_all[:, dc * P:(dc + 1) * P],
                        acc[:, dc * P:(dc + 1) * P],
                        ident[:],
                    )
                nc.scalar.copy(out=xt_all[:, :H], in_=pt_all[:, :H])
                for dc in range(NDC // 2, NDC):
                    nc.tensor.transpose(
                        pt_all[:, dc * P:(dc + 1) * P],
                        acc[:, dc * P:(dc + 1) * P],
                        ident[:],
                    )
                nc.scalar.copy(out=xt_all[:, H:], in_=pt_all[:, H:])

                po = po_pool.tile([P, K], F32)
                nc.tensor.matmul(
                    po[:], lhsT=ones_sb[:], rhs=bias_sb[:], start=True, stop=False
                )
                for dc in range(NDC):
                    nc.tensor.matmul(
                        po[:],
                        lhsT=xt_all[:, dc * P:(dc + 1) * P],
                        rhs=wt_sb[:, dc * K:(dc + 1) * K],
                        start=False,
                        stop=(dc == NDC - 1),
                    )
                return po

            # Emission order per block: adds(n) -> loads(n+1) -> psum/matmul
            # stage(n) -> tanh+y(n-1). This keeps every ACT/sync DMA trigger
            # for block n+1 AHEAD of block n's copy/tanh/y in the engine
            # FIFOs, so the two x-stream rings never stall behind compute.
            # (Putting project(n) before loads(n+1) was measured to REMOVE
            # the stabilizing transpose-governor: the stream then sprints
            # and collapses to ~337 GB/s -- do not reorder.)
            xt = issue_loads(0)
            prev_po = None
            for blk in range(NBLK):
                last = blk == NBLK - 1
                acc = reduce_last(xt) if last else reduce(xt)
                if not last:
                    xt = issue_loads(blk + 1)
                po = project_last(acc) if last else project(acc)
                if prev_po is not None:
                    finish(blk - 1, prev_po)
                prev_po = po
            finish(NBLK - 1, prev_po)

    nc.compile()
    _nc_cache = nc
    return nc


def _ensure_ntff_hook():
    """Register the axon NTFF profile hook if the image's antenv lacks it."""
    import sys
    import types

    try:
        from antenv.axon_hooks import get_axon_ntff_profile_hook  # noqa: F401
        return
    except ImportError:
        pass
    import antenv

    mod = types.ModuleType("antenv.axon_hooks")
    mod._hook = None

    def set_axon_ntff_profile_hook(h):
        mod._hook = h

    def get_axon_ntff_profile_hook():
        return mod._hook

    mod.set_axon_ntff_profile_hook = set_axon_ntff_profile_hook
    mod.get_axon_ntff_profile_hook = get_axon_ntff_profile_hook
    sys.modules["antenv.axon_hooks"] = mod
    antenv.axon_hooks = mod
    try:
        from trn_agent_boot.trn_boot import _ntff_profile_via_ctypes

        mod._hook = _ntff_profile_via_ctypes("/opt/axon/libaxon_pjrt.so")
    except Exception:
        mod._hook = None


def _run(inputs, trace=False, **kwargs):
    x = np.asarray(inputs["x"], dtype=np.float32)
    W = np.asarray(inputs["W"], dtype=np.float32)
    b = np.asarray(inputs["b"], dtype=np.float32)
    wt = np.ascontiguousarray(W.T).astype(np.float32) * np.float32(1.0 / L)
    bias = np.ascontiguousarray(b.reshape(1, K)).astype(np.float32)
    in_maps = [
        {
            "x": np.ascontiguousarray(x[:, c * BS:(c + 1) * BS, :]),
            "wt": wt,
            "bias": bias,
        }
        for c in range(NCORES)
    ]
    if trace:
        _ensure_ntff_hook()
        import concourse.bass_utils as bu

        bu.upload_artifacts = lambda tmpdir: "local://skipped"
    nc = _build()
    res = run_bass_kernel_spmd(
        nc, in_maps, core_ids=list(range(NCORES)), trace=trace, **kwargs
    )
    y = np.concatenate([r["y"] for r in res.results], axis=0)
    return y, res


def kernel(**inputs):
    y, _ = _run(inputs)
    return y


# revision 31
# speedup vs baseline: 1.1786x; 1.1248x over previous
"""Trainium2 Bass kernel for nn_BitwiseHashing.

Computes out = tanh(mean_l(x) @ W.T + b) for x:[12,8192,1024] f32,
W:[64,1024], b:[64] -> out:[8192,64].

Strategy (data-parallel over 8 NeuronCores):
  - shard x along batch dim: 1024 rows per core (48 MiB each, streamed).
  - host pre-transposes W to wt = (W.T / L) [1024,64]; bias shipped as [1,64].
  - per 128-row block: stream 12 L-slices (contiguous 512 KiB DMAs),
    accumulate with DVE adds, PE-transpose the 8 [128,128] d-chunks of the
    sum, then project against wt.  The PSUM->SBUF copy on ACT casts the
    transposed sum to bf16 for free, so the projection matmuls run
    single-pass bf16 instead of double-pass fp32 -- that shortens the
    per-block post-arrival latency, which sets the drain tail after the
    last HBM byte lands.

  Scheduling note (measured, do not "fix"): the transposes reading the
  in-place accumulator tile deliberately tie two x-load triggers per
  block to the PE pipeline.  That acts as a governor holding the stream
  at a stable ~396 GB/s.  Decoupling the accumulator into its own pool
  lets the stream sprint at 419 GB/s (the SDMA fabric cap) but the
  free-running regime is only marginally stable and collapses to
  ~338 GB/s mid-stream, which is a net loss (measured repeatedly).
"""

import numpy as np

import concourse.bacc as bacc
import concourse.mybir as mybir
from concourse import tile
from concourse.masks import make_identity
from concourse.bass_utils import run_bass_kernel_spmd

L, B, D, K = 12, 8192, 1024, 64
NCORES = 8
BS = B // NCORES      # 1024 batch rows per core
P = 128               # partitions
NBLK = BS // P        # 8 row blocks per core
NDC = D // P          # 8 contraction chunks
F32 = mybir.dt.float32
BF16 = mybir.dt.bfloat16

_nc_cache = None


def _build():
    global _nc_cache
    if _nc_cache is not None:
        return _nc_cache

    nc = bacc.Bacc("TRN2", target_bir_lowering=False, debug=False)
    x = nc.dram_tensor("x", [L, BS, D], F32, kind="ExternalInput")
    wt = nc.dram_tensor("wt", [D, K], F32, kind="ExternalInput")
    bias = nc.dram_tensor("bias", [1, K], F32, kind="ExternalInput")
    y = nc.dram_tensor("y", [BS, K], F32, kind="ExternalOutput")

    with tile.TileContext(nc) as tc:
        with (
            tc.tile_pool(name="const", bufs=1) as cpool,
            tc.tile_pool(name="xin", bufs=26) as xpool,
            tc.tile_pool(name="xt", bufs=2) as tpool,
            tc.tile_pool(name="out", bufs=3) as opool,
            tc.tile_pool(name="pt", bufs=2, space="PSUM") as pt_pool,
            tc.tile_pool(name="po", bufs=2, space="PSUM") as po_pool,
        ):
            # constants go over the SWDGE queue to keep both HWDGE rings
            # free for the x stream from t=0; the bf16 casts run on
            # gpsimd during the startup window
            wt_f32 = cpool.tile([P, NDC * K], F32)
            for dc in range(NDC):
                nc.gpsimd.dma_start(
                    out=wt_f32[:, dc * K:(dc + 1) * K],
                    in_=wt.ap()[dc * P:(dc + 1) * P, :],
                )
            bias_f32 = cpool.tile([1, K], F32)
            nc.gpsimd.dma_start(out=bias_f32[:], in_=bias.ap())
            wt_sb = cpool.tile([P, NDC * K], BF16)
            nc.gpsimd.tensor_copy(out=wt_sb[:], in_=wt_f32[:])
            bias_sb = cpool.tile([1, K], BF16)
            nc.gpsimd.tensor_copy(out=bias_sb[:], in_=bias_f32[:])
            ones_sb = cpool.tile([1, P], BF16)
            nc.gpsimd.memset(ones_sb[:], 1.0)
            ident = cpool.tile([P, P], F32)
            make_identity(nc, ident[:])
            # last-block scratch: fresh single-write destinations for the
            # tail adds -- in-place sub-range accumulation there made the
            # Tile scheduler insert an ~0.9us DVE DRAIN mid-drain
            accL = cpool.tile([P, D], F32)
            accN = cpool.tile([P, D], F32)
            acch2 = cpool.tile([P, D], F32)

            xap = x.ap()
            yap = y.ap()

            def issue_loads(blk):
                b0 = blk * P
                last = blk == NBLK - 1
                xt = []
                for l in range(L):
                    xl = xpool.tile([P, D], F32)
                    eng = nc.sync if l % 2 == 0 else nc.scalar
                    if last and l == L - 1:
                        # final tile of the whole stream: load in two
                        # halves so the tail's h0 adds can run while the
                        # h1 half is still in flight
                        eng.dma_start(
                            out=xl[:, :H], in_=xap[l, b0:b0 + P, :H]
                        )
                        eng.dma_start(
                            out=xl[:, H:], in_=xap[l, b0:b0 + P, H:]
                        )
                    else:
                        eng.dma_start(out=xl[:], in_=xap[l, b0:b0 + P, :])
                    xt.append(xl)
                return xt

            def reduce(xt):
                # two independent running chains, one per DMA ring: the
                # even tiles (sync ring) and odd tiles (scalar ring) each
                # complete in FIFO order within their ring, so each chain
                # only ever waits on its own ring and inter-ring skew
                # cannot stall the reduction
                accE, accO = xt[0], xt[1]
                for l in range(2, L, 2):
                    nc.vector.tensor_add(
                        out=accE[:], in0=accE[:], in1=xt[l][:]
                    )
                    nc.vector.tensor_add(
                        out=accO[:], in0=accO[:], in1=xt[l + 1][:]
                    )
                nc.vector.tensor_add(out=accE[:], in0=accE[:], in1=accO[:])
                return accE

            def project(acc):
                # transpose the block sum into PSUM (single-op groups),
                # then one wide PSUM->SBUF copy on ACT that also casts to
                # bf16, and the K-projection in single-pass bf16 matmuls
                pt_all = pt_pool.tile([P, D], F32)
                for dc in range(NDC):
                    nc.tensor.transpose(
                        pt_all[:, dc * P:(dc + 1) * P],
                        acc[:, dc * P:(dc + 1) * P],
                        ident[:],
                    )
                xt_all = tpool.tile([P, D], BF16)
                nc.scalar.copy(out=xt_all[:], in_=pt_all[:])

                po = po_pool.tile([P, K], F32)
                # bias broadcast across partitions: ones[1,128].T @ bias[1,64]
                nc.tensor.matmul(
                    po[:], lhsT=ones_sb[:], rhs=bias_sb[:], start=True, stop=False
                )
                for dc in range(NDC):
                    nc.tensor.matmul(
                        po[:],
                        lhsT=xt_all[:, dc * P:(dc + 1) * P],
                        rhs=wt_sb[:, dc * K:(dc + 1) * K],
                        start=False,
                        stop=(dc == NDC - 1),
                    )
                return po

            def finish(blk, po):
                b0 = blk * P
                ot = opool.tile([P, K], F32)
                nc.scalar.activation(
                    ot[:], po[:], mybir.ActivationFunctionType.Tanh
                )
                nc.sync.dma_start(out=yap[b0:b0 + P, :], in_=ot[:])

            H = D // 2

            def reduce_last(xt):
                # last block: the tail after the final HBM byte is pure
                # drain, so pipeline it in D-halves -- only ~1.2us of DVE
                # work (two half-width adds) gates the first transposes
                # instead of ~2.4us of full-width adds
                accE, accO = xt[0], xt[1]
                for l in range(2, L - 2, 2):
                    nc.vector.tensor_add(
                        out=accE[:], in0=accE[:], in1=xt[l][:]
                    )
                    nc.vector.tensor_add(
                        out=accO[:], in0=accO[:], in1=xt[l + 1][:]
                    )
                # t10/t11 + merge at half granularity, every write to a
                # fresh tile range (single-write: no WAW/DRAIN hazards).
                # The t10-only adds are hoisted ahead of the t11-gated
                # ones so they drain from the DVE queue while t11's two
                # half-loads are still in flight; after the last HBM
                # byte only accN-h/acch2-h remain on the critical path.
                h0, h1 = slice(0, H), slice(H, D)
                nc.vector.tensor_add(
                    out=accL[:, h0], in0=accE[:, h0], in1=xt[10][:, h0]
                )
                nc.vector.tensor_add(
                    out=accL[:, h1], in0=accE[:, h1], in1=xt[10][:, h1]
                )
                for h in (h0, h1):
                    nc.vector.tensor_add(
                        out=accN[:, h], in0=accO[:, h], in1=xt[11][:, h]
                    )
                    nc.vector.tensor_add(
                        out=acch2[:, h], in0=accL[:, h], in1=accN[:, h]
                    )
                return acch2

            def project_last(acc):
                # half-pipelined: transposes+copy of the low 512 columns
                # start while the high half is still merging; projection
                # matmuls for the low half overlap the high half's copy
                pt_all = pt_pool.tile([P, D], F32)
                xt_all = tpool.tile([P, D], BF16)
                for dc in range(NDC // 2):
                    nc.tensor.transpose(
                        pt_all[:, dc * P:(dc + 1) * P],
                        acc[:, dc * P:(dc + 1) * P],
                        ident[:],
                    )
                nc.scalar.copy(out=xt_all[:, :H], in_=pt_all[:, :H])
                for dc in range(NDC // 2, NDC):
                    nc.tensor.transpose(
                        pt_all[:, dc * P:(dc + 1) * P],
                        acc[:, dc * P:(dc + 1) * P],
                        ident[:],
                    )
                nc.scalar.copy(out=xt_all[:, H:], in_=pt_all[:, H:])

                po = po_pool.tile([P, K], F32)
                nc.tensor.matmul(
                    po[:], lhsT=ones_sb[:], rhs=bias_sb[:], start=True, stop=False
                )
                for dc in range(NDC):
                    nc.tensor.matmul(
                        po[:],
                        lhsT=xt_all[:, dc * P:(dc + 1) * P],
                        rhs=wt_sb[:, dc * K:(dc + 1) * K],
                        start=False,
                        stop=(dc == NDC - 1),
                    )
                return po

            # Emission order per block: adds(n) -> loads(n+1) -> psum/matmul
            # stage(n) -> tanh+y(n-1). This keeps every ACT/sync DMA trigger
            # for block n+1 AHEAD of block n's copy/tanh/y in the engine
            # FIFOs, so the two x-stream rings never stall behind compute.
            # (Putting project(n) before loads(n+1) was measured to REMOVE
            # the stabilizing transpose-governor: the stream then sprints
            # and collapses to ~337 GB/s -- do not reorder.)
            xt = issue_loads(0)
            prev_po = None
            for blk in range(NBLK):
                last = blk == NBLK - 1
                acc = reduce_last(xt) if last else reduce(xt)
                if not last:
                    xt = issue_loads(blk + 1)
                po = project_last(acc) if last else project(acc)
                if prev_po is not None:
                    finish(blk - 1, prev_po)
                prev_po = po
            finish(NBLK - 1, prev_po)

    nc.compile()
    _nc_cache = nc
    return nc


def _ensure_ntff_hook():
    """Register the axon NTFF profile hook if the image's antenv lacks it."""
    import sys
    import types

    try:
        from antenv.axon_hooks import get_axon_ntff_profile_hook  # noqa: F401
        return
    except ImportError:
        pass
    import antenv

    mod = types.ModuleType("antenv.axon_hooks")
    mod._hook = None

    def set_axon_ntff_profile_hook(h):
        mod._hook = h

    def get_axon_ntff_profile_hook():
        return mod._hook

    mod.set_axon_ntff_profile_hook = set_axon_ntff_profile_hook
    mod.get_axon_ntff_profile_hook = get_axon_ntff_profile_hook
    sys.modules["antenv.axon_hooks"] = mod
    antenv.axon_hooks = mod
    try:
        from trn_agent_boot.trn_boot import _ntff_profile_via_ctypes

        mod._hook = _ntff_profile_via_ctypes("/opt/axon/libaxon_pjrt.so")
    except Exception:
        mod._hook = None


def _run(inputs, trace=False, **kwargs):
    x = np.asarray(inputs["x"], dtype=np.float32)
    W = np.asarray(inputs["W"], dtype=np.float32)
    b = np.asarray(inputs["b"], dtype=np.float32)
    wt = np.ascontiguousarray(W.T).astype(np.float32) * np.float32(1.0 / L)
    bias = np.ascontiguousarray(b.reshape(1, K)).astype(np.float32)
    in_maps = [
        {
            "x": np.ascontiguousarray(x[:, c * BS:(c + 1) * BS, :]),
            "wt": wt,
            "bias": bias,
        }
        for c in range(NCORES)
    ]
    if trace:
        _ensure_ntff_hook()
        import concourse.bass_utils as bu

        bu.upload_artifacts = lambda tmpdir: "local://skipped"
    nc = _build()
    res = run_bass_kernel_spmd(
        nc, in_maps, core_ids=list(range(NCORES)), trace=trace, **kwargs
    )
    y = np.concatenate([r["y"] for r in res.results], axis=0)
    return y, res


def kernel(**inputs):
    y, _ = _run(inputs)
    return y
